# revision 1
# baseline (speedup 1.0000x reference)
"""TRN2 Bass kernel for nn_AttentionExample_3882650435947.

Reference math:
    enc    = encoder_outputs[:, 0, :]                      # [S, H]
    cat    = [broadcast(hidden), enc]                      # [S, 2H]
    energy = cat @ attn_W.T + attn_b                       # [S, H]
    scores = energy @ other[0]                             # [S]
    out    = softmax(scores)[None, None, :]                # [1, 1, S]

Algebraic reduction used here:
    scores = cat @ (attn_W.T @ other[0]) + attn_b . other[0]
The attn_b term and the hidden-part of cat contribute the SAME constant to
every score, and softmax is shift-invariant, so with W2 = attn_W[:, H:2H]
and v = W2.T @ other[0]:
    out = softmax(enc @ v)
exactly (in real arithmetic).  This turns a 275-GMAC matmul into two matvecs
(17 + 34 MMAC) plus a softmax, and drops hidden / attn_b / attn_W[:, :H]
from the computation entirely.

Distribution over 8 NeuronCores (hidden-dim sharding, one AllReduce):
  core r gets columns c in [r*512, (r+1)*512) of enc (host-transposed) and
  of W2.  It computes v_r = W2[:, blk].T @ other locally on the PE, then
  partial_scores[s] = sum_{c in blk} enc[s, c] * v[c] for ALL s, again on
  the PE (enc tiles are the stationary operand; weight-load bound).  One
  32 KiB AllReduce sums the partials; every core then runs the identical
  softmax over the 8192 scores and writes the full output (core 0's copy
  is returned).

Data layouts (host-prepared so every DMA is contiguous):
  encT   [512, 8192]  encT[c_local, j*128+q] = enc[s = q*64+j, r*512+c_local]
                      (s-index interleaved so the 64 PSUM score tiles land
                       in natural p-major order: scores_sb[q, j] = s=q*64+j)
  w2     [4096, 512]  attn_W[:, H + r*512 : H + (r+1)*512]
  otherp [128, 32]    otherp[p, hk] = other[0, hk*128 + p]
  out    [128, 64]    out[q, j] = softmax(scores)[q*64 + j]
"""

import numpy as np

NCORES = 8
S = 8192
H = 4096
CBLK = H // NCORES   # 512 hidden columns per core
KH = H // 128        # 32 contraction chunks for v
CT = CBLK // 128     # 4 psum tiles for v / c-chunks for scores
NJ = S // 128        # 64 score tiles
SGRP = 8             # enc DMA column groups (1024 s-columns each)

_CACHE = {}


def _build_nc():
    import concourse.mybir as mybir
    import concourse.bacc as bacc
    import concourse.tile as tile
    from concourse import bass_isa

    f32 = mybir.dt.float32
    nc = bacc.Bacc(
        "TRN2", target_bir_lowering=False, debug=False, num_devices=NCORES
    )

    encT = nc.dram_tensor("encT", [CBLK, S], f32, kind="ExternalInput")
    w2 = nc.dram_tensor("w2", [H, CBLK], f32, kind="ExternalInput")
    otherp = nc.dram_tensor("otherp", [128, KH], f32, kind="ExternalInput")
    out = nc.dram_tensor("out", [128, NJ], f32, kind="ExternalOutput")

    with tile.TileContext(nc) as tc:
        with (
            tc.tile_pool(name="sb_w2", bufs=4) as w2_pool,
            tc.tile_pool(name="sb_enc", bufs=8) as enc_pool,
            tc.tile_pool(name="sb_misc", bufs=1) as misc,
            tc.tile_pool(name="ps", bufs=4, space="PSUM") as ps,
            tc.tile_pool(name="dram", bufs=1, space="DRAM") as dram,
        ):
            other_sb = misc.tile([128, KH], f32)
            nc.sync.dma_start(other_sb[:], otherp[:, :])

            # ---- v_r = W2_blk.T @ other : contraction over h in 32 chunks ----
            vps = [
                ps.tile([128, 1], f32, tag="vps", name=f"vps{ct}")
                for ct in range(CT)
            ]
            for hk in range(KH):
                w2_t = w2_pool.tile([128, CBLK], f32, tag="w2t", name=f"w2t{hk}")
                nc.sync.dma_start(w2_t[:], w2[hk * 128 : (hk + 1) * 128, :])
                for ct in range(CT):
                    nc.tensor.matmul(
                        vps[ct][:],
                        w2_t[:, ct * 128 : (ct + 1) * 128],
                        other_sb[:, hk : hk + 1],
                        start=(hk == 0),
                        stop=(hk == KH - 1),
                    )
            v_sb = misc.tile([128, CT], f32)
            for ct in range(CT):
                nc.vector.tensor_copy(v_sb[:, ct : ct + 1], vps[ct][:])

            # ---- partial scores for all 8192 s on this core's c-block ----
            scores_sb = misc.tile([128, NJ], f32)
            for sg in range(SGRP):
                enc_ts = []
                for ck in range(CT):
                    enc_t = enc_pool.tile(
                        [128, 1024], f32, tag="enct", name=f"enc{sg}_{ck}"
                    )
                    nc.sync.dma_start(
                        enc_t[:],
                        encT[ck * 128 : (ck + 1) * 128, sg * 1024 : (sg + 1) * 1024],
                    )
                    enc_ts.append(enc_t)
                for jj in range(SGRP):
                    j = sg * SGRP + jj
                    ps_t = ps.tile([128, 1], f32, tag="ps_t", name=f"ps{j}")
                    for ck in range(CT):
                        nc.tensor.matmul(
                            ps_t[:],
                            enc_ts[ck][:, jj * 128 : (jj + 1) * 128],
                            v_sb[:, ck : ck + 1],
                            start=(ck == 0),
                            stop=(ck == CT - 1),
                        )
                    nc.any.tensor_copy(scores_sb[:, j : j + 1], ps_t[:])

            # ---- AllReduce the partial scores (32 KiB) ----
            sc_in = dram.tile([128, NJ], f32)
            sc_out = dram.tile([128, NJ], f32)
            nc.sync.dma_start(sc_in[:], scores_sb[:])
            nc.gpsimd.collective_compute(
                "AllReduce",
                mybir.AluOpType.add,
                replica_groups=[list(range(NCORES))],
                ins=[sc_in.opt()],
                outs=[sc_out.opt()],
            )

            # ---- softmax over all 8192 scores (redundant on every core) ----
            ssb = misc.tile([128, NJ], f32)
            nc.sync.dma_start(ssb[:], sc_out[:])
            m_f = misc.tile([128, 1], f32)
            nc.vector.reduce_max(m_f[:], ssb[:], axis=mybir.AxisListType.X)
            m_g = misc.tile([128, 1], f32)
            nc.gpsimd.partition_all_reduce(
                m_g[:], m_f[:], channels=128, reduce_op=bass_isa.ReduceOp.max
            )
            negm = misc.tile([128, 1], f32)
            nc.vector.tensor_scalar_mul(negm[:], m_g[:], -1.0)
            e_sb = misc.tile([128, NJ], f32)
            rowsum = misc.tile([128, 1], f32)
            nc.scalar.activation(
                e_sb[:],
                ssb[:],
                mybir.ActivationFunctionType.Exp,
                bias=negm[:],
                scale=1.0,
                accum_out=rowsum[:],
            )
            z_g = misc.tile([128, 1], f32)
            nc.gpsimd.partition_all_reduce(
                z_g[:], rowsum[:], channels=128, reduce_op=bass_isa.ReduceOp.add
            )
            invz = misc.tile([128, 1], f32)
            nc.vector.reciprocal(invz[:], z_g[:])
            attn = misc.tile([128, NJ], f32)
            nc.vector.tensor_scalar_mul(attn[:], e_sb[:], invz[:])
            nc.sync.dma_start(out[:, :], attn[:])

    nc.compile()
    return nc


def _get_nc():
    if "nc" not in _CACHE:
        _CACHE["nc"] = _build_nc()
    return _CACHE["nc"]


def make_in_maps(encoder_outputs, attn_W, other):
    enc = np.ascontiguousarray(np.asarray(encoder_outputs, dtype=np.float32)).reshape(
        S, H
    )
    W = np.asarray(attn_W, dtype=np.float32)
    oth = np.asarray(other, dtype=np.float32).reshape(H)

    # encT[c, j*128 + q] = enc[q*64 + j, c]
    encT = np.ascontiguousarray(
        enc.reshape(128, NJ, H).transpose(2, 1, 0).reshape(H, S)
    )
    w2full = W[:, H:]
    otherp = np.ascontiguousarray(oth.reshape(KH, 128).T)

    in_maps = []
    for r in range(NCORES):
        in_maps.append(
            {
                "encT": encT[r * CBLK : (r + 1) * CBLK, :],
                "w2": np.ascontiguousarray(w2full[:, r * CBLK : (r + 1) * CBLK]),
                "otherp": otherp,
            }
        )
    return in_maps


def run(encoder_outputs, attn_W, other, trace=False):
    from concourse import bass_utils

    nc = _get_nc()
    in_maps = make_in_maps(encoder_outputs, attn_W, other)
    res = bass_utils.run_bass_kernel_spmd(
        nc, in_maps, core_ids=list(range(NCORES)), trace=trace
    )
    attn = np.asarray(res.results[0]["out"], dtype=np.float32).reshape(S)
    return attn.reshape(1, 1, S), res


def kernel(hidden, encoder_outputs, attn_W, attn_b, other):
    # hidden / attn_b / attn_W[:, :H] only shift every score by the same
    # constant, which softmax ignores (see module docstring).
    out, _ = run(encoder_outputs, attn_W, other)
    return out


# revision 2
# speedup vs baseline: 1.7766x; 1.7766x over previous
"""TRN2 Bass kernel for nn_AttentionExample_3882650435947.

Reference math:
    enc    = encoder_outputs[:, 0, :]                      # [S, H]
    cat    = [broadcast(hidden), enc]                      # [S, 2H]
    energy = cat @ attn_W.T + attn_b                       # [S, H]
    scores = energy @ other[0]                             # [S]
    out    = softmax(scores)[None, None, :]                # [1, 1, S]

Algebraic reduction used here:
    scores = cat @ (attn_W.T @ other[0]) + attn_b . other[0]
The attn_b term and the hidden-part of cat contribute the SAME constant to
every score, and softmax is shift-invariant, so with W2 = attn_W[:, H:2H]
and v = W2.T @ other[0]:
    out = softmax(enc @ v)
exactly (in real arithmetic).  This turns a 275-GMAC matmul into two matvecs
(17 + 34 MMAC) plus a softmax, and drops hidden / attn_b / attn_W[:, :H]
from the computation entirely.

Distribution over 8 NeuronCores (hidden-dim sharding, one AllReduce):
  core r gets columns c in [r*512, (r+1)*512) of enc (host-transposed) and
  of W2.  It computes v_r = W2[:, blk].T @ other locally on the PE, then
  partial_scores[s] = sum_{c in blk} enc[s, c] * v[c] for ALL s, again on
  the PE (enc tiles are the stationary operand; weight-load bound).  One
  32 KiB AllReduce sums the partials; every core then runs the identical
  softmax over the 8192 scores and writes the full output (core 0's copy
  is returned).

Data layouts (host-prepared so every DMA is contiguous):
  encT   [512, 8192]  encT[c_local, j*128+q] = enc[s = q*64+j, r*512+c_local]
                      (s-index interleaved so the 64 PSUM score tiles land
                       in natural p-major order: scores_sb[q, j] = s=q*64+j)
  w2     [4096, 512]  attn_W[:, H + r*512 : H + (r+1)*512]
  otherp [128, 32]    otherp[p, hk] = other[0, hk*128 + p]
  out    [128, 64]    out[q, j] = softmax(scores)[q*64 + j]
"""

import numpy as np

NCORES = 8
S = 8192
H = 4096
CBLK = H // NCORES   # 512 hidden columns per core
KH = H // 128        # 32 contraction chunks for v
CT = CBLK // 128     # 4 psum tiles for v / c-chunks for scores
NJ = S // 128        # 64 score tiles
SGRP = 8             # enc DMA column groups (1024 s-columns each)

_CACHE = {}


def _build_nc():
    import concourse.mybir as mybir
    import concourse.bacc as bacc
    import concourse.tile as tile
    from concourse import bass_isa

    f32 = mybir.dt.float32
    bf16 = mybir.dt.bfloat16
    nc = bacc.Bacc(
        "TRN2", target_bir_lowering=False, debug=False, num_devices=NCORES
    )

    encT = nc.dram_tensor("encT", [CBLK, S], bf16, kind="ExternalInput")
    w2 = nc.dram_tensor("w2", [H, CBLK], bf16, kind="ExternalInput")
    otherp = nc.dram_tensor("otherp", [128, KH], bf16, kind="ExternalInput")
    out = nc.dram_tensor("out", [128, NJ], f32, kind="ExternalOutput")

    with tile.TileContext(nc) as tc:
        with (
            tc.tile_pool(name="sb_w2", bufs=4) as w2_pool,
            tc.tile_pool(name="sb_enc", bufs=8) as enc_pool,
            tc.tile_pool(name="sb_misc", bufs=1) as misc,
            tc.tile_pool(name="ps", bufs=4, space="PSUM") as ps,
            tc.tile_pool(name="dram", bufs=1, space="DRAM") as dram,
        ):
            other_sb = misc.tile([128, KH], bf16)
            nc.sync.dma_start(other_sb[:], otherp[:, :])

            # ---- v_r = W2_blk.T @ other : contraction over h in 32 chunks ----
            vps = [
                ps.tile([128, 1], f32, tag="vps", name=f"vps{ct}")
                for ct in range(CT)
            ]
            for hk in range(KH):
                w2_t = w2_pool.tile([128, CBLK], bf16, tag="w2t", name=f"w2t{hk}")
                nc.sync.dma_start(w2_t[:], w2[hk * 128 : (hk + 1) * 128, :])
                for ct in range(CT):
                    nc.tensor.matmul(
                        vps[ct][:],
                        w2_t[:, ct * 128 : (ct + 1) * 128],
                        other_sb[:, hk : hk + 1],
                        start=(hk == 0),
                        stop=(hk == KH - 1),
                    )
            v_sb = misc.tile([128, CT], bf16)
            for ct in range(CT):
                nc.vector.tensor_copy(v_sb[:, ct : ct + 1], vps[ct][:])

            # ---- partial scores for all 8192 s on this core's c-block ----
            scores_sb = misc.tile([128, NJ], f32)
            for sg in range(SGRP):
                enc_ts = []
                for ck in range(CT):
                    enc_t = enc_pool.tile(
                        [128, 1024], bf16, tag="enct", name=f"enc{sg}_{ck}"
                    )
                    nc.sync.dma_start(
                        enc_t[:],
                        encT[ck * 128 : (ck + 1) * 128, sg * 1024 : (sg + 1) * 1024],
                    )
                    enc_ts.append(enc_t)
                for jj in range(SGRP):
                    j = sg * SGRP + jj
                    ps_t = ps.tile([128, 1], f32, tag="ps_t", name=f"ps{j}")
                    for ck in range(CT):
                        nc.tensor.matmul(
                            ps_t[:],
                            enc_ts[ck][:, jj * 128 : (jj + 1) * 128],
                            v_sb[:, ck : ck + 1],
                            start=(ck == 0),
                            stop=(ck == CT - 1),
                        )
                    nc.any.tensor_copy(scores_sb[:, j : j + 1], ps_t[:])

            # ---- AllReduce the partial scores (32 KiB) ----
            sc_in = dram.tile([128, NJ], f32)
            sc_out = dram.tile([128, NJ], f32)
            nc.sync.dma_start(sc_in[:], scores_sb[:])
            nc.gpsimd.collective_compute(
                "AllReduce",
                mybir.AluOpType.add,
                replica_groups=[list(range(NCORES))],
                ins=[sc_in.opt()],
                outs=[sc_out.opt()],
            )

            # ---- softmax over all 8192 scores (redundant on every core) ----
            ssb = misc.tile([128, NJ], f32)
            nc.sync.dma_start(ssb[:], sc_out[:])
            m_f = misc.tile([128, 1], f32)
            nc.vector.reduce_max(m_f[:], ssb[:], axis=mybir.AxisListType.X)
            m_g = misc.tile([128, 1], f32)
            nc.gpsimd.partition_all_reduce(
                m_g[:], m_f[:], channels=128, reduce_op=bass_isa.ReduceOp.max
            )
            negm = misc.tile([128, 1], f32)
            nc.vector.tensor_scalar_mul(negm[:], m_g[:], -1.0)
            e_sb = misc.tile([128, NJ], f32)
            rowsum = misc.tile([128, 1], f32)
            nc.scalar.activation(
                e_sb[:],
                ssb[:],
                mybir.ActivationFunctionType.Exp,
                bias=negm[:],
                scale=1.0,
                accum_out=rowsum[:],
            )
            z_g = misc.tile([128, 1], f32)
            nc.gpsimd.partition_all_reduce(
                z_g[:], rowsum[:], channels=128, reduce_op=bass_isa.ReduceOp.add
            )
            invz = misc.tile([128, 1], f32)
            nc.vector.reciprocal(invz[:], z_g[:])
            attn = misc.tile([128, NJ], f32)
            nc.vector.tensor_scalar_mul(attn[:], e_sb[:], invz[:])
            nc.sync.dma_start(out[:, :], attn[:])

    nc.compile()
    return nc


def _get_nc():
    if "nc" not in _CACHE:
        _CACHE["nc"] = _build_nc()
    return _CACHE["nc"]


def make_in_maps(encoder_outputs, attn_W, other):
    import ml_dtypes

    bf = ml_dtypes.bfloat16
    enc = np.asarray(encoder_outputs, dtype=np.float32).reshape(S, H).astype(bf)
    W = np.asarray(attn_W, dtype=np.float32)
    oth = np.asarray(other, dtype=np.float32).reshape(H).astype(bf)

    # encT[c, j*128 + q] = enc[q*64 + j, c]
    encT = np.ascontiguousarray(
        enc.reshape(128, NJ, H).transpose(2, 1, 0).reshape(H, S)
    )
    w2full = W[:, H:].astype(bf)
    otherp = np.ascontiguousarray(oth.reshape(KH, 128).T)

    in_maps = []
    for r in range(NCORES):
        in_maps.append(
            {
                "encT": encT[r * CBLK : (r + 1) * CBLK, :],
                "w2": np.ascontiguousarray(w2full[:, r * CBLK : (r + 1) * CBLK]),
                "otherp": otherp,
            }
        )
    return in_maps


def run(encoder_outputs, attn_W, other, trace=False):
    from concourse import bass_utils

    nc = _get_nc()
    in_maps = make_in_maps(encoder_outputs, attn_W, other)
    res = bass_utils.run_bass_kernel_spmd(
        nc, in_maps, core_ids=list(range(NCORES)), trace=trace
    )
    attn = np.asarray(res.results[0]["out"], dtype=np.float32).reshape(S)
    return attn.reshape(1, 1, S), res


def kernel(hidden, encoder_outputs, attn_W, attn_b, other):
    # hidden / attn_b / attn_W[:, :H] only shift every score by the same
    # constant, which softmax ignores (see module docstring).
    out, _ = run(encoder_outputs, attn_W, other)
    return out


# revision 4
# speedup vs baseline: 2.2583x; 1.2711x over previous
"""TRN2 Bass kernel for nn_AttentionExample_3882650435947.

Reference math:
    enc    = encoder_outputs[:, 0, :]                      # [S, H]
    cat    = [broadcast(hidden), enc]                      # [S, 2H]
    energy = cat @ attn_W.T + attn_b                       # [S, H]
    scores = energy @ other[0]                             # [S]
    out    = softmax(scores)[None, None, :]                # [1, 1, S]

Algebraic reduction used here:
    scores = cat @ (attn_W.T @ other[0]) + attn_b . other[0]
The attn_b term and the hidden-part of cat contribute the SAME constant to
every score, and softmax is shift-invariant, so with W2 = attn_W[:, H:2H]
and v = W2.T @ other[0]:
    out = softmax(enc @ v)
exactly (in real arithmetic).  This turns a 275-GMAC matmul into two matvecs
(17 + 34 MMAC) plus a softmax, and drops hidden / attn_b / attn_W[:, :H]
from the computation entirely.

Distribution over 8 NeuronCores (hidden-dim sharding, one AllReduce):
  core r gets columns c in [r*512, (r+1)*512) of enc (host-transposed) and
  of W2.  It computes v_r = W2[:, blk].T @ other locally on the PE, then
  partial_scores[s] = sum_{c in blk} enc[s, c] * v[c] for ALL s, again on
  the PE (enc tiles are the stationary operand; weight-load bound).  One
  32 KiB AllReduce sums the partials; every core then runs the identical
  softmax over the 8192 scores and writes the full output (core 0's copy
  is returned).

Data layouts (host-prepared so every DMA is contiguous):
  encT   [512, 8192]  encT[c_local, j*128+q] = enc[s = q*64+j, r*512+c_local]
                      (s-index interleaved so the 64 PSUM score tiles land
                       in natural p-major order: scores_sb[q, j] = s=q*64+j)
  w2     [4096, 512]  attn_W[:, H + r*512 : H + (r+1)*512]
  otherp [128, 32]    otherp[p, hk] = other[0, hk*128 + p]
  out    [128, 64]    out[q, j] = softmax(scores)[q*64 + j]
"""

import numpy as np

NCORES = 8
S = 8192
H = 4096
CBLK = H // NCORES   # 512 hidden columns per core
KH = H // 128        # 32 contraction chunks for v
CT = CBLK // 128     # 4 psum tiles for v / c-chunks for scores
NJ = S // 128        # 64 score tiles
SGRP = 8             # enc DMA column groups (1024 s-columns each)

_CACHE = {}


def _build_nc():
    import concourse.mybir as mybir
    import concourse.bacc as bacc
    import concourse.tile as tile
    from concourse import bass_isa

    f32 = mybir.dt.float32
    bf16 = mybir.dt.bfloat16
    nc = bacc.Bacc(
        "TRN2", target_bir_lowering=False, debug=False, num_devices=NCORES
    )

    encT = nc.dram_tensor("encT", [CBLK, S], bf16, kind="ExternalInput")
    w2 = nc.dram_tensor("w2", [H, CBLK], bf16, kind="ExternalInput")
    otherp = nc.dram_tensor("otherp", [128, KH], bf16, kind="ExternalInput")
    out = nc.dram_tensor("out", [128, NJ], f32, kind="ExternalOutput")

    with tile.TileContext(nc) as tc:
        with (
            tc.tile_pool(name="sb_w2", bufs=4) as w2_pool,
            tc.tile_pool(name="sb_enc", bufs=4) as enc_pool,
            tc.tile_pool(name="sb_misc", bufs=1) as misc,
            tc.tile_pool(name="ps", bufs=4, space="PSUM") as ps,
            tc.tile_pool(name="dram", bufs=1, space="DRAM") as dram,
        ):
            other_sb = misc.tile([128, KH], bf16)
            nc.sync.dma_start(other_sb[:], otherp[:, :])

            # Grouped DRAM views so each dma_start moves 0.5-1 MiB (the
            # HWDGE issue cost on the Sync sequencer is ~0.6 us per
            # dma_start, so many small DMAs serialize on issue).
            # w2_view[g][p, sub, c'] = w2[(4g+sub)*128 + p, c']
            w2_view = w2.ap().rearrange("(g s p) c -> g p s c", s=4, p=128)
            # enc_view[p, ck, s] = encT[ck*128 + p, s]
            enc_view = encT.ap().rearrange("(ck p) s -> p ck s", p=128)

            # ---- v_r = W2_blk.T @ other : contraction over h in 32 chunks ----
            vps = [
                ps.tile([128, 1], f32, tag="vps", name=f"vps{ct}")
                for ct in range(CT)
            ]
            for g in range(KH // 4):
                w2_t = w2_pool.tile([128, 4, CBLK], bf16, tag="w2t", name=f"w2t{g}")
                nc.sync.dma_start(w2_t[:], w2_view[g])
                for sub in range(4):
                    hk = g * 4 + sub
                    for ct in range(CT):
                        nc.tensor.matmul(
                            vps[ct][:],
                            w2_t[:, sub, ct * 128 : (ct + 1) * 128],
                            other_sb[:, hk : hk + 1],
                            start=(hk == 0),
                            stop=(hk == KH - 1),
                        )
            v_sb = misc.tile([128, CT], bf16)
            for ct in range(CT):
                nc.vector.tensor_copy(v_sb[:, ct : ct + 1], vps[ct][:])

            # ---- partial scores for all 8192 s on this core's c-block ----
            scores_sb = misc.tile([128, NJ], f32)
            for sg in range(SGRP):
                enc_t = enc_pool.tile(
                    [128, CT, 1024], bf16, tag="enct", name=f"enc{sg}"
                )
                nc.sync.dma_start(
                    enc_t[:], enc_view[:, :, sg * 1024 : (sg + 1) * 1024]
                )
                for jj in range(SGRP):
                    j = sg * SGRP + jj
                    ps_t = ps.tile([128, 1], f32, tag="ps_t", name=f"ps{j}")
                    for ck in range(CT):
                        nc.tensor.matmul(
                            ps_t[:],
                            enc_t[:, ck, jj * 128 : (jj + 1) * 128],
                            v_sb[:, ck : ck + 1],
                            start=(ck == 0),
                            stop=(ck == CT - 1),
                        )
                    nc.vector.tensor_copy(scores_sb[:, j : j + 1], ps_t[:])

            # ---- AllReduce the partial scores (32 KiB) ----
            sc_in = dram.tile([128, NJ], f32)
            sc_out = dram.tile([128, NJ], f32)
            nc.sync.dma_start(sc_in[:], scores_sb[:])
            nc.gpsimd.collective_compute(
                "AllReduce",
                mybir.AluOpType.add,
                replica_groups=[list(range(NCORES))],
                ins=[sc_in.opt()],
                outs=[sc_out.opt()],
            )

            # ---- softmax over all 8192 scores (redundant on every core) ----
            ssb = misc.tile([128, NJ], f32)
            nc.sync.dma_start(ssb[:], sc_out[:])
            m_f = misc.tile([128, 1], f32)
            nc.vector.reduce_max(m_f[:], ssb[:], axis=mybir.AxisListType.X)
            m_g = misc.tile([128, 1], f32)
            nc.gpsimd.partition_all_reduce(
                m_g[:], m_f[:], channels=128, reduce_op=bass_isa.ReduceOp.max
            )
            negm = misc.tile([128, 1], f32)
            nc.vector.tensor_scalar_mul(negm[:], m_g[:], -1.0)
            e_sb = misc.tile([128, NJ], f32)
            rowsum = misc.tile([128, 1], f32)
            nc.scalar.activation(
                e_sb[:],
                ssb[:],
                mybir.ActivationFunctionType.Exp,
                bias=negm[:],
                scale=1.0,
                accum_out=rowsum[:],
            )
            z_g = misc.tile([128, 1], f32)
            nc.gpsimd.partition_all_reduce(
                z_g[:], rowsum[:], channels=128, reduce_op=bass_isa.ReduceOp.add
            )
            invz = misc.tile([128, 1], f32)
            nc.vector.reciprocal(invz[:], z_g[:])
            attn = misc.tile([128, NJ], f32)
            nc.vector.tensor_scalar_mul(attn[:], e_sb[:], invz[:])
            nc.sync.dma_start(out[:, :], attn[:])

    nc.compile()
    return nc


def _get_nc():
    if "nc" not in _CACHE:
        _CACHE["nc"] = _build_nc()
    return _CACHE["nc"]


def make_in_maps(encoder_outputs, attn_W, other):
    import ml_dtypes

    bf = ml_dtypes.bfloat16
    enc = np.asarray(encoder_outputs, dtype=np.float32).reshape(S, H).astype(bf)
    W = np.asarray(attn_W, dtype=np.float32)
    oth = np.asarray(other, dtype=np.float32).reshape(H).astype(bf)

    # encT[c, j*128 + q] = enc[q*64 + j, c]
    encT = np.ascontiguousarray(
        enc.reshape(128, NJ, H).transpose(2, 1, 0).reshape(H, S)
    )
    w2full = W[:, H:].astype(bf)
    otherp = np.ascontiguousarray(oth.reshape(KH, 128).T)

    in_maps = []
    for r in range(NCORES):
        in_maps.append(
            {
                "encT": encT[r * CBLK : (r + 1) * CBLK, :],
                "w2": np.ascontiguousarray(w2full[:, r * CBLK : (r + 1) * CBLK]),
                "otherp": otherp,
            }
        )
    return in_maps


def run(encoder_outputs, attn_W, other, trace=False):
    from concourse import bass_utils

    nc = _get_nc()
    in_maps = make_in_maps(encoder_outputs, attn_W, other)
    res = bass_utils.run_bass_kernel_spmd(
        nc, in_maps, core_ids=list(range(NCORES)), trace=trace
    )
    attn = np.asarray(res.results[0]["out"], dtype=np.float32).reshape(S)
    return attn.reshape(1, 1, S), res


def kernel(hidden, encoder_outputs, attn_W, attn_b, other):
    # hidden / attn_b / attn_W[:, :H] only shift every score by the same
    # constant, which softmax ignores (see module docstring).
    out, _ = run(encoder_outputs, attn_W, other)
    return out
